# revision 15
# baseline (speedup 1.0000x reference)
"""Trainium2 Bass kernel for nn_MultiHeadAttention_558345748575.

Sharding: data-parallel over batch B=8 across the 8 NeuronCores (one batch
element per core, full weights replicated).

Per-core math (batch b, N=1024 tokens, D=512):
  ctsT = W_cts.T @ x.T           [H*L, N]   (heads along rows, 64 rows each)
  nghT = W_ngh.T @ x.T           [H*L, N]
  v    = x @ W_com               [N, HS] (+ ones column -> [N, HS+1])
  per head h, per row-block i:
     w[i, j]  = ctsT[h].T @ nghT[h]          (K=64)
     P        = exp(w)                        (no max-subtract; |w| <~ 10)
     zT[h]   += v~[i].T @ P                   -> [HS+1, N] in PSUM
  row 64 of zT[h] = per-head column sums of P; d[j] = sum_h -> softmax denom.
  g[j, :]  = sum_h zT[h][0:64, j].T @ W_grp[h]   (unnormalized y @ W_grp)
  y[j, :]  = g[j, :] / d[j]
  MLP computed transposed: h_{l+1}^T = relu(W_l.T @ h_l^T + b_l) so the bias
  is per-partition; final layer back in [token, feat] orientation.

Matmul operands use dt.float32r (~19-bit mantissa fast path, 1 cyc/row);
transposes and non-matmul tensors stay fp32.
"""

import os
import numpy as np
from contextlib import ExitStack

_SKIP = set(filter(None, os.environ.get("KSKIP", "").split(",")))

B, N, D_IN, L, H, HS, D_OUT, HID = 8, 1024, 512, 64, 8, 64, 512, 256
NCORES = 8
NCH = N // 128  # 8 token chunks of 128
DCH = D_IN // 128  # 4 feature chunks


def _build_module():
    import concourse.bacc as bacc
    import concourse.tile as tile
    from concourse import mybir

    f32 = mybir.dt.float32
    f32r = mybir.dt.float32r
    AF = mybir.ActivationFunctionType

    nc = bacc.Bacc("TRN2", target_bir_lowering=False, debug=False,
                   num_devices=NCORES)

    x_d = nc.dram_tensor("x", [N, D_IN], f32, kind="ExternalInput").ap()
    Wcts_d = nc.dram_tensor("Wcts", [D_IN, H * L], f32, kind="ExternalInput").ap()
    Wngh_d = nc.dram_tensor("Wngh", [D_IN, H * L], f32, kind="ExternalInput").ap()
    Wcom_d = nc.dram_tensor("Wcom", [D_IN, HS], f32, kind="ExternalInput").ap()
    Wgrp_d = nc.dram_tensor("Wgrp", [H * HS, D_OUT], f32, kind="ExternalInput").ap()
    W0_d = nc.dram_tensor("W0", [D_OUT + D_IN, HID], f32, kind="ExternalInput").ap()
    W1_d = nc.dram_tensor("W1", [HID, HID], f32, kind="ExternalInput").ap()
    W2_d = nc.dram_tensor("W2", [HID, HID], f32, kind="ExternalInput").ap()
    W3_d = nc.dram_tensor("W3", [HID, HID], f32, kind="ExternalInput").ap()
    W4_d = nc.dram_tensor("W4", [HID, D_IN], f32, kind="ExternalInput").ap()
    bc0_d = nc.dram_tensor("bc0", [128, 2], f32, kind="ExternalInput").ap()
    bc1_d = nc.dram_tensor("bc1", [128, 2], f32, kind="ExternalInput").ap()
    bc2_d = nc.dram_tensor("bc2", [128, 2], f32, kind="ExternalInput").ap()
    bc3_d = nc.dram_tensor("bc3", [128, 2], f32, kind="ExternalInput").ap()
    b4bc_d = nc.dram_tensor("b4bc", [128, D_IN], f32, kind="ExternalInput").ap()
    ident_d = nc.dram_tensor("ident", [128, 128], f32, kind="ExternalInput").ap()
    onescol_d = nc.dram_tensor("onescol", [128, 1], f32, kind="ExternalInput").ap()
    out_d = nc.dram_tensor("out", [N, D_IN], f32, kind="ExternalOutput").ap()

    with tile.TileContext(nc) as tc, ExitStack() as ctx:
        const = ctx.enter_context(tc.tile_pool(name="const", bufs=1))
        persist = ctx.enter_context(tc.tile_pool(name="persist", bufs=1))

        # ---- constants / weights -------------------------------------------
        ident = const.tile([128, 128], f32, name="ident_sb", tag="ident_sb")
        nc.sync.dma_start(ident[:], ident_d[:])
        ones_col = const.tile([128, 1], f32, name="ones_col", tag="ones_col")
        nc.sync.dma_start(ones_col[:], onescol_d[:])
        actwarm = const.tile([1, 2], f32, name="actwarm", tag="actwarm")
        nc.vector.memset(actwarm[:], 0.0)
        # warm the ACT exp table (~2.7us) during the DMA prologue
        nc.scalar.activation(actwarm[:], actwarm[:], AF.Exp)

        def load_f32r(name, dram_ap, rows, cols, pool=persist):
            tiles = []
            for i in range(rows // 128):
                t = pool.tile([128, cols], f32r, name=f"{name}{i}",
                              tag=f"{name}{i}")
                nc.gpsimd.dma_start(t[:], dram_ap[i * 128:(i + 1) * 128, :])
                tiles.append(t)
            return tiles

        Wcom_sb = load_f32r("Wcom", Wcom_d, D_IN, HS)
        # per-head 64-row tiles so lhsT/rhs base partitions match in P7
        Wgrp_sb = []
        for h in range(H):
            t = persist.tile([HS, D_OUT], f32r, name=f"Wgrp{h}", tag=f"Wgrp{h}")
            nc.gpsimd.dma_start(t[:], Wgrp_d[h * HS:(h + 1) * HS, :])
            Wgrp_sb.append(t)
        W0_sb = load_f32r("W0", W0_d, D_OUT + D_IN, HID)
        W1_sb = load_f32r("W1", W1_d, HID, HID)
        W2_sb = load_f32r("W2", W2_d, HID, HID)
        W3_sb = load_f32r("W3", W3_d, HID, HID)
        W4_sb = load_f32r("W4", W4_d, HID, D_IN)
        bc_sb = []
        for l, bd in enumerate((bc0_d, bc1_d, bc2_d, bc3_d)):
            t = const.tile([128, 2], f32, name=f"bc{l}", tag=f"bc{l}")
            nc.sync.dma_start(t[:], bd[:])
            bc_sb.append(t)
        b4bc = const.tile([128, D_IN], f32, name="b4bc_sb", tag="b4bc_sb")
        nc.sync.dma_start(b4bc[:], b4bc_d[:])

        # ---- persistent activations ----------------------------------------
        xT_sb = [persist.tile([128, N], f32r, name=f"xT{i}", tag=f"xT{i}")
                 for i in range(DCH)]
        v_sb = [persist.tile([128, HS + 1], f32r, name=f"v{i}", tag=f"v{i}")
                for i in range(NCH)]
        zT_sb = [persist.tile([HS + 1, N], f32r, name=f"zT{h}", tag=f"zT{h}")
                 for h in range(H)]
        yT_sb = [persist.tile([128, N], f32r, name=f"yT{i}", tag=f"yT{i}")
                 for i in range(DCH)]
        rd_rect = persist.tile([128, NCH], f32, name="rd_rect", tag="rd_rect")

        # ---- P1: x -> SBUF, transpose to xT --------------------------------
        with tc.tile_pool(name="xpool", bufs=1) as xpool, \
             tc.tile_pool(name="ps1", bufs=2, space="PSUM") as ps1:
            x_sb = []
            for i in range(NCH):
                t = xpool.tile([128, D_IN], f32, name=f"x_sb{i}", tag=f"x_sb{i}")
                nc.sync.dma_start(t[:], x_d[i * 128:(i + 1) * 128, :])
                x_sb.append(t)
            for dc in range(DCH):
                for nh in range(2):
                    pst = ps1.tile([128, 512], f32, name="xtp", tag="xtp")
                    for k in range(4):
                        nck = nh * 4 + k
                        nc.tensor.transpose(
                            pst[:, k * 128:(k + 1) * 128],
                            x_sb[nck][:, dc * 128:(dc + 1) * 128],
                            ident[:],
                        )
                    nc.vector.tensor_copy(
                        xT_sb[dc][:, nh * 512:(nh + 1) * 512], pst[:])

        # ---- P2+P3: encoders (ctsT/nghT) + v, then attention ---------------
        with tc.tile_pool(name="encpool", bufs=1) as encpool:
            Wcts_sb = load_f32r("Wcts", Wcts_d, D_IN, H * L, pool=encpool)
            Wngh_sb = load_f32r("Wngh", Wngh_d, D_IN, H * L, pool=encpool)
            ctsT_sb = [encpool.tile([128, N], f32r, name=f"ctsT{i}",
                                    tag=f"ctsT{i}") for i in range(DCH)]
            nghT_sb = [encpool.tile([128, N], f32r, name=f"nghT{i}",
                                    tag=f"nghT{i}") for i in range(DCH)]

            with tc.tile_pool(name="ps2", bufs=2, space="PSUM") as ps2:
                for wsb, enc_out in ((Wcts_sb, ctsT_sb), (Wngh_sb, nghT_sb)):
                    for cc in range(DCH):
                        for nh in range(2):
                            pse = ps2.tile([128, 512], f32, name="enc", tag="enc")
                            for dc in range(DCH):
                                nc.tensor.matmul(
                                    pse[:],
                                    wsb[dc][:, cc * 128:(cc + 1) * 128],
                                    xT_sb[dc][:, nh * 512:(nh + 1) * 512],
                                    start=(dc == 0), stop=(dc == DCH - 1),
                                )
                            nc.vector.tensor_copy(
                                enc_out[cc][:, nh * 512:(nh + 1) * 512], pse[:])

            with tc.tile_pool(name="ps3", bufs=2, space="PSUM") as ps3:
                for ic in range(NCH):
                    psv = ps3.tile([128, HS], f32, name="vps", tag="vps")
                    for dc in range(DCH):
                        nc.tensor.matmul(
                            psv[:],
                            xT_sb[dc][:, ic * 128:(ic + 1) * 128],
                            Wcom_sb[dc][:],
                            start=(dc == 0), stop=(dc == DCH - 1),
                        )
                    nc.scalar.copy(v_sb[ic][:, 0:HS], psv[:])
                    nc.gpsimd.dma_start(v_sb[ic][:, HS:HS + 1], onescol_d[:])

            # ---- P4: attention ---------------------------------------------
            with tc.tile_pool(name="ppool", bufs=3) as ppool, \
                 tc.tile_pool(name="ps4", bufs=2, space="PSUM") as ps4:
                for h in range(H):
                    ct = ctsT_sb[h // 2]
                    ng = nghT_sb[h // 2]
                    ro = 64 * (h % 2)
                    zps = ps4.tile([HS + 1, N], f32, name="zps", tag="zps")
                    for ic in range(NCH):
                        wps = ps4.tile([128, N], f32, name="wps", tag="wps")
                        for jh in range(2):
                            nc.tensor.matmul(
                                wps[:, jh * 512:(jh + 1) * 512],
                                ct[ro:ro + 64, ic * 128:(ic + 1) * 128],
                                ng[ro:ro + 64, jh * 512:(jh + 1) * 512],
                                start=True, stop=True,
                            )
                        pt = ppool.tile([128, N], f32r, name="pt", tag="pt")
                        nc.scalar.activation(pt[:], wps[:], AF.Exp)
                        for jh in range(2):
                            nc.tensor.matmul(
                                zps[:, jh * 512:(jh + 1) * 512],
                                v_sb[ic][:],
                                pt[:, jh * 512:(jh + 1) * 512],
                                start=(ic == 0), stop=(ic == NCH - 1),
                            )
                    nc.vector.tensor_copy(zT_sb[h][:], zps[:])

        # ---- P5+P6: softmax denominator -> rd_rect -------------------------
        if "P5" in _SKIP:
            nc.vector.memset(rd_rect[:], 1.0)
        else:
            # d[j] (softmax denominator) directly in column form:
            # dcol[j, jc] += zT[h][ones-row, j-chunk jc].T @ ones
            with tc.tile_pool(name="ps5", bufs=1, space="PSUM") as ps5:
                dps = ps5.tile([128, NCH], f32, name="dps", tag="dps")
                for jc in range(NCH):
                    for h in range(H):
                        nc.tensor.matmul(
                            dps[:, jc:jc + 1],
                            zT_sb[h][HS:HS + 1,
                                     jc * 128:(jc + 1) * 128].bitcast(f32),
                            ones_col[HS:HS + 1, 0:1],
                            start=(h == 0), stop=(h == H - 1),
                        )
                nc.vector.reciprocal(rd_rect[:], dps[:])

        # ---- P7: g = z @ Wgrp, scale by 1/d -> y; P8: y -> yT --------------
        with tc.tile_pool(name="ypool", bufs=1) as ypool:
            y_sb = [ypool.tile([128, D_OUT], f32, name=f"y{i}", tag=f"y{i}")
                    for i in range(NCH)]
            with tc.tile_pool(name="ps7", bufs=2, space="PSUM") as ps7:
                for jc in range(NCH):
                    psg = ps7.tile([128, D_OUT], f32, name="gps", tag="gps")
                    for h in range(H):
                        nc.tensor.matmul(
                            psg[:],
                            zT_sb[h][0:HS, jc * 128:(jc + 1) * 128],
                            Wgrp_sb[h][:],
                            start=(h == 0), stop=(h == H - 1),
                        )
                    nc.scalar.activation(y_sb[jc][:], psg[:], AF.Copy,
                                         scale=rd_rect[:, jc:jc + 1])

            with tc.tile_pool(name="ps8", bufs=2, space="PSUM") as ps8:
                for oc in range(DCH):
                    for nh in range(2):
                        pst = ps8.tile([128, 512], f32, name="ytp", tag="ytp")
                        for k in range(4):
                            jc = nh * 4 + k
                            nc.tensor.transpose(
                                pst[:, k * 128:(k + 1) * 128],
                                y_sb[jc][:, oc * 128:(oc + 1) * 128],
                                ident[:],
                            )
                        nc.vector.tensor_copy(
                            yT_sb[oc][:, nh * 512:(nh + 1) * 512], pst[:])

        # ---- P9: MLP layers 0-3 (transposed) -------------------------------
        with tc.tile_pool(name="hpool", bufs=1) as hpool, \
             tc.tile_pool(name="ps9", bufs=2, space="PSUM") as ps9:
            rhs_tiles = xT_sb + yT_sb
            h_prev = None
            for lyr, (wsb, bcol) in enumerate(
                    ((W0_sb, bc_sb[0]), (W1_sb, bc_sb[1]),
                     (W2_sb, bc_sb[2]), (W3_sb, bc_sb[3]))):
                h_next = [hpool.tile([128, N], f32r, name=f"h{lyr}_{c}",
                                     tag=f"h{lyr}_{c}") for c in range(2)]
                for cc in range(2):
                    for nh in range(2):
                        psm = ps9.tile([128, 512], f32, name="mlp", tag="mlp")
                        for k, kt in enumerate(rhs_tiles):
                            nc.tensor.matmul(
                                psm[:],
                                wsb[k][:, cc * 128:(cc + 1) * 128],
                                kt[:, nh * 512:(nh + 1) * 512],
                                start=(k == 0), stop=(k == len(rhs_tiles) - 1),
                            )
                        nc.scalar.activation(
                            h_next[cc][:, nh * 512:(nh + 1) * 512], psm[:],
                            AF.Relu, bias=bcol[:, cc:cc + 1])
                rhs_tiles = h_next
                h_prev = h_next

            # ---- P10: final layer, [token, feat] orientation ---------------
            with tc.tile_pool(name="opool", bufs=3) as opool:
                for jc in range(NCH):
                    pso = ps9.tile([128, D_IN], f32, name="out_ps", tag="out_ps")
                    for k in range(2):
                        nc.tensor.matmul(
                            pso[:],
                            h_prev[k][:, jc * 128:(jc + 1) * 128],
                            W4_sb[k][:],
                            start=(k == 0), stop=(k == 1),
                        )
                    osb = opool.tile([128, D_IN], f32, name="osb", tag="osb")
                    nc.vector.tensor_add(osb[:], pso[:], b4bc[:])
                    nc.sync.dma_start(out_d[jc * 128:(jc + 1) * 128, :], osb[:])

    nc.compile()
    return nc


def _make_in_maps(inputs):
    g = lambda k: np.ascontiguousarray(np.asarray(inputs[k], dtype=np.float32))
    x = g("x")
    common = {
        "Wcts": g("W_cts"), "Wngh": g("W_ngh"), "Wcom": g("W_com"),
        "Wgrp": g("W_grp"),
        "W0": g("W0"), "W1": g("W1"), "W2": g("W2"), "W3": g("W3"),
        "W4": g("W4"),
        "b4bc": np.ascontiguousarray(
            np.broadcast_to(g("b4"), (128, D_IN))),
        "ident": np.eye(128, dtype=np.float32),
        "onescol": np.ones((128, 1), dtype=np.float32),
    }
    for l in range(4):
        b = g(f"b{l}")  # [256] -> [128, 2] column form
        common[f"bc{l}"] = np.ascontiguousarray(b.reshape(2, 128).T)
    return [{**common, "x": np.ascontiguousarray(x[b])} for b in range(B)]


_NC_CACHE = []


def _get_module():
    if not _NC_CACHE:
        _NC_CACHE.append(_build_module())
    return _NC_CACHE[0]


def run_on_hw(inputs, **kw):
    from concourse import bass_utils
    nc = _get_module()
    in_maps = _make_in_maps(inputs)
    res = bass_utils.run_bass_kernel_spmd(
        nc, in_maps, core_ids=list(range(NCORES)), **kw)
    out = np.stack([np.asarray(res.results[b]["out"]) for b in range(B)], 0)
    return out.astype(np.float32), res


def kernel(**inputs) -> np.ndarray:
    out, _ = run_on_hw(inputs)
    return out
